# revision 11
# baseline (speedup 1.0000x reference)
"""NT-Xent loss kernel for 8 Trainium2 NeuronCores (Bass/Tile).

Strategy (data-parallel over rows, mirrors the GatherLayer path):
  - host: z = concat(z_i, z_j) -> [16384, 256] f32; each core gets the full z
    (replicated) plus its 2048-row slice and the matching positive-partner
    slice.
  - device (identical SPMD IR on all 8 cores):
      * normalize full z row-wise in fp32 (sumsq on DVE, rnorm = exp(-ln/2)
        on ACT - same activation table set as the main Exp), cast bf16,
        transpose on the tensor engine (bf16 PSUM tiles) into per-group
        zn^T tiles so the main loop can start before the whole preamble ends.
      * flash-style loop over 1024-col PSUM groups: K=256 via 2 accumulated
        bf16 matmuls per 512-col slice; one ACT Exp per group; row-sums
        fused via ACT accum_out on even groups and GPSIMD tensor_reduce on
        odd groups (keeps ACT near pure-exp throughput).
      * per-row diagonal dot (bf16, matches the PE's diagonal term) and
        positive-pair dot (fp32) on DVE.
  - host: lse_i = log(expsum_i - exp(10*diag_i)) in fp64, pos_i = 10*posdot_i,
    loss = mean(lse - pos).
"""

import os
import numpy as np

try:
    import concourse.bass as bass
except ImportError:  # pragma: no cover
    import sys

    sys.path.insert(0, "/opt/trn_rl_repo")
    import concourse.bass as bass

import concourse.mybir as mybir
import concourse.tile as tile
from concourse.bass_utils import run_bass_kernel_spmd

F32 = mybir.dt.float32
BF16 = mybir.dt.bfloat16

B = 8192
D = 256
N = 2 * B  # 16384
NCORES = 8
RPC = N // NCORES  # 2048 rows per core
RT = RPC // 128  # 16 local row tiles
GB = 8  # z_full load/transpose groups (16 tiles each)
GW = N // GB  # 2048 columns of znT per group tile
CG = 16  # main-loop column groups
CW = N // CG  # 1024 cols per PSUM group (2 banks)
SUB = 512  # matmul free dim (1 PSUM bank)
TEMP_INV = 10.0  # 1 / temperature
EPS2 = 1e-16  # cos eps^2 (clamp on squared norm)

# set by the last run when BASS_TRACE=1 (read by test.py)
last_exec_time_ns = None
last_mean_exec_time_ns = None

_CACHE = {}


def _fixup_bir(bir_bytes):
    """Adapt Tile-emitted BIR to this container's walrus build:
    - split instructions carrying >1 sync wait (walrus allows one per inst)
    - replace the raw-ISA EVENT_SEMAPHORE_RANGE_CLEAR (encoding mismatch)
      with per-semaphore sem-wr-imm zero writes."""
    import json

    b = json.loads(bir_bytes)
    for fn in b["functions"]:
        for blk in fn["blocks"]:
            new_ins = []
            for ins in blk["instructions"]:
                if (
                    ins.get("opcode") == "ISA"
                    and ins.get("op_name") == "EVENT_SEMAPHORE_RANGE_CLEAR"
                ):
                    d = ins["ant_dict"]
                    for s in range(d["range_first"], d["range_last"] + 1):
                        new_ins.append(
                            {
                                "debug": ins.get("debug", 0),
                                "engine": ins["engine"],
                                "ins": [],
                                "outs": [],
                                "name": f'{ins["name"]}_z{s}',
                                "opcode": "EventSemaphore",
                                "sync_info": {
                                    "on_update": [
                                        {
                                            "ant_name": f"zero_{s}",
                                            "id": s,
                                            "sync_type": "semaphore",
                                            "update_mode": "sem-wr-imm",
                                            "update_value": 0,
                                        }
                                    ],
                                    "on_wait": [],
                                },
                            }
                        )
                    continue
                si = ins.get("sync_info")
                if si:
                    waits = si.get("on_wait") or []
                    if len(waits) > 1:
                        for j, w in enumerate(waits[:-1]):
                            new_ins.append(
                                {
                                    "debug": ins.get("debug", 0),
                                    "engine": ins["engine"],
                                    "ins": [],
                                    "outs": [],
                                    "name": f'{ins["name"]}_w{j}',
                                    "opcode": "EventSemaphore",
                                    "sync_info": {"on_update": [], "on_wait": [w]},
                                }
                            )
                        si["on_wait"] = [waits[-1]]
                new_ins.append(ins)
            blk["instructions"] = new_ins
    return json.dumps(b).encode()


_PATCHED = False


def _install_bir_fixup():
    """Route the pjrt compile path's BIR bytes through _fixup_bir."""
    global _PATCHED
    if _PATCHED:
        return
    from concourse import bass2jax

    orig = bass2jax._decompress_ant_bir

    def patched(ant_bir_value):
        return _fixup_bir(orig(ant_bir_value))

    bass2jax._decompress_ant_bir = patched
    _PATCHED = True


def _rnorm(nc, pool, ss, tag):
    """ss [128, k] squared norms -> 1/max(sqrt(ss), eps) = exp(-0.5*ln(ss)).

    Ln+Exp live in the same activation table set as the main-loop Exp, so
    no ACT table reloads (Sqrt would force a set switch per group)."""
    ln = pool.tile(list(ss.shape), F32, tag=tag + "_ln")
    rn = pool.tile(list(ss.shape), F32, tag=tag + "_rn")
    nc.vector.tensor_scalar_max(out=ss, in0=ss, scalar1=EPS2)
    nc.scalar.activation(out=ln, in_=ss, func=mybir.ActivationFunctionType.Ln)
    nc.scalar.activation(
        out=rn, in_=ln, func=mybir.ActivationFunctionType.Exp, scale=-0.5
    )
    return rn


def _sumsq(nc, pool, a, b, accum_col):
    """accum_col [128,1] = sum over free dim of a*b (fp32), two DVE ops."""
    s = pool.tile([128, D], F32, tag="sq")
    nc.vector.tensor_mul(s, a, b)
    nc.vector.reduce_sum(out=accum_col, in_=s, axis=mybir.AxisListType.X)


def _emit(tc, nc, z_full, z_loc, z_pos, ident_in, out):
    from contextlib import ExitStack

    Exp = mybir.ActivationFunctionType.Exp
    X = mybir.AxisListType.X

    with ExitStack() as ctx:
        singles = ctx.enter_context(tc.tile_pool(name="singles", bufs=1))
        zbig = ctx.enter_context(tc.tile_pool(name="zbig", bufs=2))
        znb = ctx.enter_context(tc.tile_pool(name="znb", bufs=40))
        sq = ctx.enter_context(tc.tile_pool(name="sq", bufs=3))
        st = ctx.enter_context(tc.tile_pool(name="st", bufs=3))
        esp = ctx.enter_context(tc.tile_pool(name="es", bufs=3))
        otp = ctx.enter_context(tc.tile_pool(name="ot", bufs=3))
        mmp = ctx.enter_context(tc.tile_pool(name="mmp", bufs=3, space="PSUM"))
        tpp = ctx.enter_context(tc.tile_pool(name="tpp", bufs=2, space="PSUM"))

        ident = singles.tile([128, 128], BF16)
        nc.sync.dma_start(out=ident, in_=ident_in[:, :])

        znT0 = [singles.tile([128, GW], BF16, name=f"znT0_{i}", tag=f"znT0_{i}") for i in range(GB)]
        znT1 = [singles.tile([128, GW], BF16, name=f"znT1_{i}", tag=f"znT1_{i}") for i in range(GB)]
        znTl0 = singles.tile([128, RPC], BF16)
        znTl1 = singles.tile([128, RPC], BF16)
        diag = singles.tile([128, RT], F32)
        posd = singles.tile([128, RT], F32)
        rawp = singles.tile([128, RT], F32)
        acc3 = singles.tile([128, RT, CG], F32)

        def load_block(src):
            zb = zbig.tile([128, RT, D], F32, tag="zb")
            nc.sync.dma_start(
                out=zb, in_=src.rearrange("(k p) d -> p k d", p=128)
            )
            return zb

        def transpose_group(zn_tiles, dst0, dst1):
            """zn_tiles: 16 [128, D] bf16 tiles -> dst0/dst1 [128, 2048] bf16
            (d-chunk-major transposed layout) via PE transposes + DVE copies."""
            for half, dst in ((0, dst0), (1, dst1)):
                for quarter in range(2):
                    tp = tpp.tile([128, 1024], BF16, tag="tp")
                    for j in range(8):
                        t = quarter * 8 + j
                        nc.tensor.transpose(
                            tp[:, j * 128 : (j + 1) * 128],
                            zn_tiles[t][:, half * 128 : (half + 1) * 128],
                            ident,
                        )
                    nc.vector.tensor_copy(
                        dst[:, quarter * 1024 : (quarter + 1) * 1024], tp
                    )

        # ---- local rows + positive partners
        zl = load_block(z_loc[:, :])
        zp = load_block(z_pos[:, :])
        ss_l = st.tile([128, RT], F32, tag="ss_l")
        ss_p = st.tile([128, RT], F32, tag="ss_p")
        for t in range(RT):
            _sumsq(nc, sq, zl[:, t, :], zl[:, t, :], ss_l[:, t : t + 1])
            _sumsq(nc, sq, zp[:, t, :], zp[:, t, :], ss_p[:, t : t + 1])
            _sumsq(nc, sq, zl[:, t, :], zp[:, t, :], rawp[:, t : t + 1])
        rn_l = _rnorm(nc, st, ss_l, "l")
        rn_p = _rnorm(nc, st, ss_p, "p")
        # posd = rawp * rn_l * rn_p  (fp32 cosine of positive pairs)
        nc.vector.tensor_mul(posd, rawp, rn_l)
        nc.vector.tensor_mul(posd, posd, rn_p)
        znl = []
        for t in range(RT):
            zb16 = znb.tile([128, D], BF16, tag="znl")
            nc.vector.tensor_scalar_mul(
                out=zb16, in0=zl[:, t, :], scalar1=rn_l[:, t : t + 1]
            )
            znl.append(zb16)
            # diagonal term exactly as the PE will compute it (bf16 inputs)
            _sumsq(nc, sq, zb16, zb16, diag[:, t : t + 1])
        transpose_group(znl, znTl0, znTl1)

        # ---- full z, pipelined per 16-tile group with the main loop
        for gb in range(GB):
            zf = load_block(z_full[gb * GW : (gb + 1) * GW, :])
            ssf = st.tile([128, RT], F32, tag="ssf")
            for t in range(RT):
                _sumsq(nc, sq, zf[:, t, :], zf[:, t, :], ssf[:, t : t + 1])
            rnf = _rnorm(nc, st, ssf, "f")
            znf = []
            for t in range(RT):
                zb16 = znb.tile([128, D], BF16, tag="znf")
                nc.vector.tensor_scalar_mul(
                    out=zb16, in0=zf[:, t, :], scalar1=rnf[:, t : t + 1]
                )
                znf.append(zb16)
            transpose_group(znf, znT0[gb], znT1[gb])

            # ---- main loop for the two 1024-col groups this gb provides
            for cg in (2 * gb, 2 * gb + 1):
                off = (cg * CW) % GW
                for r in range(RT):
                    ps = mmp.tile([128, CW], F32, tag="ps")
                    for k in range(2):
                        lhsT = (znTl0 if k == 0 else znTl1)[
                            :, r * 128 : (r + 1) * 128
                        ]
                        rhsT = (znT0 if k == 0 else znT1)[gb]
                        for s in range(CW // SUB):
                            c0 = off + s * SUB
                            nc.tensor.matmul(
                                ps[:, s * SUB : (s + 1) * SUB],
                                lhsT=lhsT,
                                rhs=rhsT[:, c0 : c0 + SUB],
                                start=(k == 0),
                                stop=(k == 1),
                            )
                    es = esp.tile([128, CW], BF16, tag="es")
                    nc.scalar.activation(
                        out=es,
                        in_=ps,
                        func=Exp,
                        scale=TEMP_INV,
                        accum_out=acc3[:, r, cg : cg + 1],
                    )

        # ---- finalize per-row outputs
        for r in range(RT):
            o = otp.tile([128, 4], F32)
            nc.vector.reduce_sum(out=o[:, 0:1], in_=acc3[:, r, :], axis=X)
            nc.vector.tensor_copy(o[:, 1:2], diag[:, r : r + 1])
            nc.vector.tensor_copy(o[:, 2:3], posd[:, r : r + 1])
            nc.vector.memset(o[:, 3:4], 0.0)
            nc.sync.dma_start(out=out[r * 128 : (r + 1) * 128, :], in_=o)


def build_program():
    if "nc" in _CACHE:
        return _CACHE["nc"]
    nc = bass.Bass()
    z_full = nc.declare_dram_parameter("z_full", [N, D], F32, isOutput=False)
    z_loc = nc.declare_dram_parameter("z_loc", [RPC, D], F32, isOutput=False)
    z_pos = nc.declare_dram_parameter("z_pos", [RPC, D], F32, isOutput=False)
    ident = nc.declare_dram_parameter("ident", [128, 128], BF16, isOutput=False)
    out = nc.declare_dram_parameter("out", [RPC, 4], F32, isOutput=True)
    with tile.TileContext(nc) as tc:
        _emit(tc, nc, z_full[:, :], z_loc[:, :], z_pos[:, :], ident[:, :], out[:, :])
    _CACHE["nc"] = nc
    return nc


def make_in_maps(z):
    import ml_dtypes

    eye = np.eye(128, dtype=ml_dtypes.bfloat16)
    in_maps = []
    for c in range(NCORES):
        r0 = c * RPC
        p0 = (r0 + B) % N
        in_maps.append(
            {
                "z_full": z,
                "z_loc": z[r0 : r0 + RPC],
                "z_pos": z[p0 : p0 + RPC],
                "ident": eye,
            }
        )
    return in_maps


def finalize(outs):
    """outs: list of [RPC, 4] arrays per core -> scalar loss (fp64 host math)."""
    o = np.concatenate(outs, axis=0).astype(np.float64)  # [N, 4]
    expsum = o[:, 0] - np.exp(TEMP_INV * o[:, 1])  # drop self-similarity term
    lse = np.log(expsum)
    pos = TEMP_INV * o[:, 2]
    return np.float32(np.mean(lse - pos))


def _enable_axon_trace_hook():
    """Best-effort: register the NTFF profile hook that the image's antenv
    stub does not ship, and neuter the artifact upload (no bucket creds
    in this container). Only needed when profiling (BASS_TRACE=1)."""
    import sys
    import types

    try:
        from antenv import axon_hooks  # noqa: F401
    except ImportError:
        try:
            import antenv
            from trn_agent_boot.trn_boot import _ntff_profile_via_ctypes

            mod = types.ModuleType("antenv.axon_hooks")
            _hook = [None]
            mod.set_axon_ntff_profile_hook = lambda h: _hook.__setitem__(0, h)
            mod.get_axon_ntff_profile_hook = lambda: _hook[0]
            sys.modules["antenv.axon_hooks"] = mod
            antenv.axon_hooks = mod
            mod.set_axon_ntff_profile_hook(
                _ntff_profile_via_ctypes("/opt/axon/libaxon_pjrt.so")
            )
        except Exception as e:  # pragma: no cover
            print(f"trace hook setup failed: {e}")
    try:
        from concourse import bass_utils as _bu

        _bu.upload_artifacts = lambda tmpdir: f"local:{tmpdir}"
    except Exception:
        pass


def kernel(z_i, z_j, logit_scale_m=None, **_unused):
    global last_exec_time_ns, last_mean_exec_time_ns
    z_i = np.ascontiguousarray(np.asarray(z_i, dtype=np.float32))
    z_j = np.ascontiguousarray(np.asarray(z_j, dtype=np.float32))
    assert z_i.shape == (B, D) and z_j.shape == (B, D)
    z = np.concatenate([z_i, z_j], axis=0)

    nc = build_program()
    in_maps = make_in_maps(z)
    _install_bir_fixup()
    trace = bool(os.environ.get("BASS_TRACE"))
    if trace:
        _enable_axon_trace_hook()
    res = run_bass_kernel_spmd(nc, in_maps, list(range(NCORES)), trace=trace)
    last_exec_time_ns = res.exec_time_ns
    last_mean_exec_time_ns = res.mean_exec_time_ns
    outs = [res.results[c]["out"] for c in range(NCORES)]
    return np.asarray(finalize(outs), dtype=np.float32)


# revision 12
# speedup vs baseline: 2.0276x; 2.0276x over previous
"""NT-Xent loss kernel for 8 Trainium2 NeuronCores (Bass/Tile).

Symmetric data-parallel strategy (each unordered pair computed once):
  - host: z = concat(z_i, z_j) [16384, 256] f32. Core c receives z rotated by
    its row offset: rot_c[i] = z[(2048c + i) % 16384]. With that rotation the
    IR is identical across cores: local rows are rot rows [0, 2048) and the
    core's column window is rot rows [0, 8192) - each unordered pair {i, j}
    lands in exactly one core's (local rows x window) block (pairs at offset
    exactly 8192 - the positive pairs - are excluded and handled on host).
  - device (identical SPMD IR on all 8 cores):
      * normalize window rows in fp32 (sumsq on DVE, rnorm = exp(-ln/2) on
        ACT - same table set as the main Exp), cast bf16, transpose on the
        tensor engine into per-group zn^T tiles (group-pipelined with the
        main loop). zn^T group 0 doubles as the local lhsT.
      * main loop over 8 x 1024-col PSUM groups x 16 local row tiles:
        2x2 accumulated bf16 matmuls (K=256) -> one ACT Exp per tile with
        fused row-sum (accum_out) -> two ones-matmuls on the PE accumulate
        the block's column sums in PSUM across the 16 row tiles (these are
        the partner rows' sums, by symmetry).
      * per-row diagonal dot (bf16, matches the PE diagonal) and fp32
        positive-pair dot on DVE.
  - host (fp64): expsum[i] = own rowsum + the 4 covering cores' colsums
    + exp(10*pos_i) - exp(10*diag_i) - bf16(exp(10*diag_i));
    loss = mean(log(expsum) - 10*pos).
"""

import os
import numpy as np

try:
    import concourse.bass as bass
except ImportError:  # pragma: no cover
    import sys

    sys.path.insert(0, "/opt/trn_rl_repo")
    import concourse.bass as bass

import concourse.mybir as mybir
import concourse.tile as tile
from concourse.bass_utils import run_bass_kernel_spmd

F32 = mybir.dt.float32
BF16 = mybir.dt.bfloat16

B = 8192
D = 256
N = 2 * B  # 16384
NCORES = 8
RPC = N // NCORES  # 2048 local rows per core
RT = RPC // 128  # 16 local row tiles
W = N // 2  # 8192-column window per core
GB = 4  # window load/transpose groups (16 tiles each)
GW = W // GB  # 2048 columns of znT per group tile
CG = 8  # main-loop column groups
CW = W // CG  # 1024 cols per PSUM group (2 banks)
SUB = 512  # matmul free dim (1 PSUM bank)
TEMP_INV = 10.0  # 1 / temperature
EPS2 = 1e-16  # cos eps^2 (clamp on squared norm)

# set by the last run when BASS_TRACE=1 (read by test.py)
last_exec_time_ns = None
last_mean_exec_time_ns = None

_CACHE = {}


def _fixup_bir(bir_bytes):
    """Adapt Tile-emitted BIR to this container's walrus build:
    - split instructions carrying >1 sync wait (walrus allows one per inst)
    - replace the raw-ISA EVENT_SEMAPHORE_RANGE_CLEAR (encoding mismatch)
      with per-semaphore sem-wr-imm zero writes."""
    import json

    b = json.loads(bir_bytes)
    for fn in b["functions"]:
        for blk in fn["blocks"]:
            new_ins = []
            for ins in blk["instructions"]:
                if (
                    ins.get("opcode") == "ISA"
                    and ins.get("op_name") == "EVENT_SEMAPHORE_RANGE_CLEAR"
                ):
                    d = ins["ant_dict"]
                    for s in range(d["range_first"], d["range_last"] + 1):
                        new_ins.append(
                            {
                                "debug": ins.get("debug", 0),
                                "engine": ins["engine"],
                                "ins": [],
                                "outs": [],
                                "name": f'{ins["name"]}_z{s}',
                                "opcode": "EventSemaphore",
                                "sync_info": {
                                    "on_update": [
                                        {
                                            "ant_name": f"zero_{s}",
                                            "id": s,
                                            "sync_type": "semaphore",
                                            "update_mode": "sem-wr-imm",
                                            "update_value": 0,
                                        }
                                    ],
                                    "on_wait": [],
                                },
                            }
                        )
                    continue
                si = ins.get("sync_info")
                if si:
                    waits = si.get("on_wait") or []
                    if len(waits) > 1:
                        for j, w in enumerate(waits[:-1]):
                            new_ins.append(
                                {
                                    "debug": ins.get("debug", 0),
                                    "engine": ins["engine"],
                                    "ins": [],
                                    "outs": [],
                                    "name": f'{ins["name"]}_w{j}',
                                    "opcode": "EventSemaphore",
                                    "sync_info": {"on_update": [], "on_wait": [w]},
                                }
                            )
                        si["on_wait"] = [waits[-1]]
                new_ins.append(ins)
            blk["instructions"] = new_ins
    return json.dumps(b).encode()


_PATCHED = False


def _install_bir_fixup():
    """Route the pjrt compile path's BIR bytes through _fixup_bir."""
    global _PATCHED
    if _PATCHED:
        return
    from concourse import bass2jax

    orig = bass2jax._decompress_ant_bir

    def patched(ant_bir_value):
        return _fixup_bir(orig(ant_bir_value))

    bass2jax._decompress_ant_bir = patched
    _PATCHED = True


def _rnorm(nc, pool, ss, tag):
    """ss [128, k] squared norms -> 1/max(sqrt(ss), eps) = exp(-0.5*ln(ss)).

    Ln+Exp live in the same activation table set as the main-loop Exp, so
    no ACT table reloads (Sqrt would force a set switch per group)."""
    ln = pool.tile(list(ss.shape), F32, tag=tag + "_ln")
    rn = pool.tile(list(ss.shape), F32, tag=tag + "_rn")
    nc.vector.tensor_scalar_max(out=ss, in0=ss, scalar1=EPS2)
    nc.scalar.activation(out=ln, in_=ss, func=mybir.ActivationFunctionType.Ln)
    nc.scalar.activation(
        out=rn, in_=ln, func=mybir.ActivationFunctionType.Exp, scale=-0.5
    )
    return rn


def _dot(nc, pool, a, b, accum_col):
    """accum_col [128,1] = sum over free dim of a*b (fp32), two DVE ops."""
    s = pool.tile([128, D], F32, tag="sq")
    nc.vector.tensor_mul(s, a, b)
    nc.vector.reduce_sum(out=accum_col, in_=s, axis=mybir.AxisListType.X)


def _emit(tc, nc, z_win, z_pos, ident_in, out, out_c):
    from contextlib import ExitStack

    Exp = mybir.ActivationFunctionType.Exp
    X = mybir.AxisListType.X

    with ExitStack() as ctx:
        singles = ctx.enter_context(tc.tile_pool(name="singles", bufs=1))
        zbig = ctx.enter_context(tc.tile_pool(name="zbig", bufs=2))
        znb = ctx.enter_context(tc.tile_pool(name="znb", bufs=40))
        sq = ctx.enter_context(tc.tile_pool(name="sq", bufs=3))
        st = ctx.enter_context(tc.tile_pool(name="st", bufs=3))
        esp = ctx.enter_context(tc.tile_pool(name="es", bufs=3))
        otp = ctx.enter_context(tc.tile_pool(name="ot", bufs=3))
        mmp = ctx.enter_context(tc.tile_pool(name="mmp", bufs=2, space="PSUM"))
        csp = ctx.enter_context(tc.tile_pool(name="csp", bufs=1, space="PSUM"))
        tpp = ctx.enter_context(tc.tile_pool(name="tpp", bufs=2, space="PSUM"))

        ident = singles.tile([128, 128], BF16)
        nc.sync.dma_start(out=ident, in_=ident_in[:, :])
        ones = singles.tile([128, 128], BF16)
        nc.vector.memset(ones, 1.0)

        znT0 = [
            singles.tile([128, GW], BF16, name=f"znT0_{i}", tag=f"znT0_{i}")
            for i in range(GB)
        ]
        znT1 = [
            singles.tile([128, GW], BF16, name=f"znT1_{i}", tag=f"znT1_{i}")
            for i in range(GB)
        ]
        diag = singles.tile([128, RT], F32)
        posd = singles.tile([128, RT], F32)
        rawp = singles.tile([128, RT], F32)
        rnl = singles.tile([128, RT], F32)
        racc = singles.tile([128, RT, CG], F32)
        csb = singles.tile([1, W], F32)

        def load_block(src):
            zb = zbig.tile([128, RT, D], F32, tag="zb")
            nc.sync.dma_start(out=zb, in_=src.rearrange("(k p) d -> p k d", p=128))
            return zb

        def transpose_group(zn_tiles, dst0, dst1):
            """16 [128, D] bf16 tiles -> dst0/dst1 [128, 2048] bf16 (d-chunk
            transposed layout) via PE transposes + DVE copies."""
            for half, dst in ((0, dst0), (1, dst1)):
                for q in range(2):
                    tp = tpp.tile([128, 1024], BF16, tag="tp")
                    for j in range(8):
                        t = q * 8 + j
                        nc.tensor.transpose(
                            tp[:, j * 128 : (j + 1) * 128],
                            zn_tiles[t][:, half * 128 : (half + 1) * 128],
                            ident,
                        )
                    nc.vector.tensor_copy(dst[:, q * 1024 : (q + 1) * 1024], tp)

        # ---- positive partners (fp32 pos dot only)
        zp = load_block(z_pos[:, :])
        ss_p = st.tile([128, RT], F32, tag="ss_p")
        for t in range(RT):
            _dot(nc, sq, zp[:, t, :], zp[:, t, :], ss_p[:, t : t + 1])
        rn_p = _rnorm(nc, st, ss_p, "p")

        # ---- window: normalize + transpose, pipelined with the main loop
        for gb in range(GB):
            zf = load_block(z_win[gb * GW : (gb + 1) * GW, :])
            ssf = st.tile([128, RT], F32, tag="ssf")
            for t in range(RT):
                _dot(nc, sq, zf[:, t, :], zf[:, t, :], ssf[:, t : t + 1])
            rnf = _rnorm(nc, st, ssf, "f")
            znf = []
            for t in range(RT):
                zb16 = znb.tile([128, D], BF16, tag="znf")
                nc.vector.tensor_scalar_mul(
                    out=zb16, in0=zf[:, t, :], scalar1=rnf[:, t : t + 1]
                )
                znf.append(zb16)
            if gb == 0:
                # group 0 = the local rows: diag dots (bf16, PE-parity),
                # raw pos dots (fp32), and keep rnorm for posd scaling
                nc.vector.tensor_copy(rnl, rnf)
                for t in range(RT):
                    _dot(nc, sq, znf[t], znf[t], diag[:, t : t + 1])
                    _dot(nc, sq, zf[:, t, :], zp[:, t, :], rawp[:, t : t + 1])
            transpose_group(znf, znT0[gb], znT1[gb])

            # ---- main loop for the two 1024-col groups this gb provides
            for cg in (2 * gb, 2 * gb + 1):
                off = (cg * CW) % GW
                cs = csp.tile([128, CW], F32, tag="cs")
                for r in range(RT):
                    ps = mmp.tile([128, CW], F32, tag="ps")
                    for k in range(2):
                        lhsT = (znT0 if k == 0 else znT1)[0][
                            :, r * 128 : (r + 1) * 128
                        ]
                        rhsT = (znT0 if k == 0 else znT1)[gb]
                        for s in range(CW // SUB):
                            c0 = off + s * SUB
                            nc.tensor.matmul(
                                ps[:, s * SUB : (s + 1) * SUB],
                                lhsT=lhsT,
                                rhs=rhsT[:, c0 : c0 + SUB],
                                start=(k == 0),
                                stop=(k == 1),
                            )
                    es = esp.tile([128, CW], BF16, tag="es")
                    nc.scalar.activation(
                        out=es,
                        in_=ps,
                        func=Exp,
                        scale=TEMP_INV,
                        accum_out=racc[:, r, cg : cg + 1],
                    )
                    # column sums (the partner rows' exp sums, by symmetry):
                    # accumulate over the 16 row tiles in PSUM
                    for s in range(CW // SUB):
                        nc.tensor.matmul(
                            cs[:, s * SUB : (s + 1) * SUB],
                            lhsT=ones,
                            rhs=es[:, s * SUB : (s + 1) * SUB],
                            start=(r == 0),
                            stop=(r == RT - 1),
                        )
                nc.vector.tensor_copy(
                    csb[0:1, cg * CW : (cg + 1) * CW], cs[0:1, :]
                )

        # ---- finalize per-row outputs
        # posd = rawp * rnl * rn_p  (fp32 cosine of positive pairs)
        nc.vector.tensor_mul(posd, rawp, rnl)
        nc.vector.tensor_mul(posd, posd, rn_p)
        for r in range(RT):
            o = otp.tile([128, 4], F32)
            nc.vector.reduce_sum(out=o[:, 0:1], in_=racc[:, r, :], axis=X)
            nc.vector.tensor_copy(o[:, 1:2], diag[:, r : r + 1])
            nc.vector.tensor_copy(o[:, 2:3], posd[:, r : r + 1])
            nc.vector.memset(o[:, 3:4], 0.0)
            nc.sync.dma_start(out=out[r * 128 : (r + 1) * 128, :], in_=o)
        nc.sync.dma_start(out=out_c[:, :], in_=csb)


def build_program():
    if "nc" in _CACHE:
        return _CACHE["nc"]
    nc = bass.Bass()
    z_win = nc.declare_dram_parameter("z_win", [W, D], F32, isOutput=False)
    z_pos = nc.declare_dram_parameter("z_pos", [RPC, D], F32, isOutput=False)
    ident = nc.declare_dram_parameter("ident", [128, 128], BF16, isOutput=False)
    out = nc.declare_dram_parameter("out", [RPC, 4], F32, isOutput=True)
    out_c = nc.declare_dram_parameter("out_c", [1, W], F32, isOutput=True)
    with tile.TileContext(nc) as tc:
        _emit(tc, nc, z_win[:, :], z_pos[:, :], ident[:, :], out[:, :], out_c[:, :])
    _CACHE["nc"] = nc
    return nc


def make_in_maps(z):
    import ml_dtypes

    eye = np.eye(128, dtype=ml_dtypes.bfloat16)
    zz = np.concatenate([z, z], axis=0)  # easy wraparound slicing
    in_maps = []
    for c in range(NCORES):
        r0 = c * RPC
        p0 = (r0 + B) % N
        in_maps.append(
            {
                "z_win": np.ascontiguousarray(zz[r0 : r0 + W]),
                "z_pos": zz[p0 : p0 + RPC],
                "ident": eye,
            }
        )
    return in_maps


def finalize(row_outs, col_outs):
    """row_outs: per-core [RPC, 4]; col_outs: per-core [1, W] -> scalar loss."""
    import ml_dtypes

    o = np.concatenate(row_outs, axis=0).astype(np.float64)  # [N, 4]
    rowsum, diagd, posd = o[:, 0], o[:, 1], o[:, 2]
    expsum = rowsum.copy()
    for c in range(NCORES):
        idx = (c * RPC + np.arange(W)) % N
        np.add.at(expsum, idx, col_outs[c].reshape(-1).astype(np.float64))
    ediag = np.exp(TEMP_INV * diagd)
    ediag_bf16 = ediag.astype(np.float32).astype(ml_dtypes.bfloat16).astype(np.float64)
    expsum += np.exp(TEMP_INV * posd) - ediag - ediag_bf16
    lse = np.log(expsum)
    return np.float32(np.mean(lse - TEMP_INV * posd))


def _enable_axon_trace_hook():
    """Best-effort: register the NTFF profile hook that the image's antenv
    stub does not ship, and neuter the artifact upload (no bucket creds
    in this container). Only needed when profiling (BASS_TRACE=1)."""
    import sys
    import types

    try:
        from antenv import axon_hooks  # noqa: F401
    except ImportError:
        try:
            import antenv
            from trn_agent_boot.trn_boot import _ntff_profile_via_ctypes

            mod = types.ModuleType("antenv.axon_hooks")
            _hook = [None]
            mod.set_axon_ntff_profile_hook = lambda h: _hook.__setitem__(0, h)
            mod.get_axon_ntff_profile_hook = lambda: _hook[0]
            sys.modules["antenv.axon_hooks"] = mod
            antenv.axon_hooks = mod
            mod.set_axon_ntff_profile_hook(
                _ntff_profile_via_ctypes("/opt/axon/libaxon_pjrt.so")
            )
        except Exception as e:  # pragma: no cover
            print(f"trace hook setup failed: {e}")
    try:
        from concourse import bass_utils as _bu

        _bu.upload_artifacts = lambda tmpdir: f"local:{tmpdir}"
    except Exception:
        pass


def kernel(z_i, z_j, logit_scale_m=None, **_unused):
    global last_exec_time_ns, last_mean_exec_time_ns
    z_i = np.ascontiguousarray(np.asarray(z_i, dtype=np.float32))
    z_j = np.ascontiguousarray(np.asarray(z_j, dtype=np.float32))
    assert z_i.shape == (B, D) and z_j.shape == (B, D)
    z = np.concatenate([z_i, z_j], axis=0)

    nc = build_program()
    in_maps = make_in_maps(z)
    _install_bir_fixup()
    trace = bool(os.environ.get("BASS_TRACE"))
    if trace:
        _enable_axon_trace_hook()
    res = run_bass_kernel_spmd(nc, in_maps, list(range(NCORES)), trace=trace)
    last_exec_time_ns = res.exec_time_ns
    last_mean_exec_time_ns = res.mean_exec_time_ns
    row_outs = [res.results[c]["out"] for c in range(NCORES)]
    col_outs = [res.results[c]["out_c"] for c in range(NCORES)]
    return np.asarray(finalize(row_outs, col_outs), dtype=np.float32)


# revision 17
# speedup vs baseline: 2.2042x; 1.0871x over previous
"""NT-Xent loss kernel for 8 Trainium2 NeuronCores (Bass/Tile).

Symmetric data-parallel strategy (each unordered pair computed once):
  - host: z = concat(z_i, z_j) [16384, 256] f32. Core c receives z rotated by
    its row offset: rot_c[i] = z[(2048c + i) % 16384]. With that rotation the
    IR is identical across cores: local rows are rot rows [0, 2048) and the
    core's column window is rot rows [0, 8192) - each unordered pair {i, j}
    lands in exactly one core's (local rows x window) block (pairs at offset
    exactly 8192 - the positive pairs - are excluded and handled on host).
  - device (identical SPMD IR on all 8 cores):
      * normalize window rows in fp32 (sumsq on DVE, rnorm = exp(-ln/2) on
        ACT - same table set as the main Exp), cast bf16, transpose on the
        tensor engine into per-group zn^T tiles (group-pipelined with the
        main loop). zn^T group 0 doubles as the local lhsT.
      * main loop over 8 x 1024-col PSUM groups x 16 local row tiles:
        2x2 accumulated bf16 matmuls (K=256) -> one ACT Exp per tile with
        fused row-sum (accum_out) -> two ones-matmuls on the PE accumulate
        the block's column sums in PSUM across the 16 row tiles (these are
        the partner rows' sums, by symmetry).
      * per-row diagonal dot (bf16, matches the PE diagonal) and fp32
        positive-pair dot on DVE.
  - host (fp64): expsum[i] = own rowsum + the 4 covering cores' colsums
    + exp(10*pos_i) - exp(10*diag_i) - bf16(exp(10*diag_i));
    loss = mean(log(expsum) - 10*pos).
"""

import os
import numpy as np

try:
    import concourse.bass as bass
except ImportError:  # pragma: no cover
    import sys

    sys.path.insert(0, "/opt/trn_rl_repo")
    import concourse.bass as bass

import concourse.mybir as mybir
import concourse.tile as tile
from concourse.bass_utils import run_bass_kernel_spmd

F32 = mybir.dt.float32
BF16 = mybir.dt.bfloat16

B = 8192
D = 256
N = 2 * B  # 16384
NCORES = 8
RPC = N // NCORES  # 2048 local rows per core
RT = RPC // 128  # 16 local row tiles
W = N // 2  # 8192-column window per core
GB = 4  # window load/transpose groups (16 tiles each)
GW = W // GB  # 2048 columns of znT per group tile
CG = 8  # main-loop column groups
CW = W // CG  # 1024 cols per PSUM group (2 banks)
SUB = 512  # matmul free dim (1 PSUM bank)
TEMP_INV = 10.0  # 1 / temperature
EPS2 = 1e-16  # cos eps^2 (clamp on squared norm)

# set by the last run when BASS_TRACE=1 (read by test.py)
last_exec_time_ns = None
last_mean_exec_time_ns = None

_CACHE = {}


def _fixup_bir(bir_bytes):
    """Adapt Tile-emitted BIR to this container's walrus build:
    - split instructions carrying >1 sync wait (walrus allows one per inst)
    - replace the raw-ISA EVENT_SEMAPHORE_RANGE_CLEAR (encoding mismatch)
      with per-semaphore sem-wr-imm zero writes."""
    import json

    b = json.loads(bir_bytes)
    for fn in b["functions"]:
        for blk in fn["blocks"]:
            new_ins = []
            for ins in blk["instructions"]:
                if (
                    ins.get("opcode") == "ISA"
                    and ins.get("op_name") == "EVENT_SEMAPHORE_RANGE_CLEAR"
                ):
                    d = ins["ant_dict"]
                    for s in range(d["range_first"], d["range_last"] + 1):
                        new_ins.append(
                            {
                                "debug": ins.get("debug", 0),
                                "engine": ins["engine"],
                                "ins": [],
                                "outs": [],
                                "name": f'{ins["name"]}_z{s}',
                                "opcode": "EventSemaphore",
                                "sync_info": {
                                    "on_update": [
                                        {
                                            "ant_name": f"zero_{s}",
                                            "id": s,
                                            "sync_type": "semaphore",
                                            "update_mode": "sem-wr-imm",
                                            "update_value": 0,
                                        }
                                    ],
                                    "on_wait": [],
                                },
                            }
                        )
                    continue
                si = ins.get("sync_info")
                if si:
                    waits = si.get("on_wait") or []
                    if len(waits) > 1:
                        for j, w in enumerate(waits[:-1]):
                            new_ins.append(
                                {
                                    "debug": ins.get("debug", 0),
                                    "engine": ins["engine"],
                                    "ins": [],
                                    "outs": [],
                                    "name": f'{ins["name"]}_w{j}',
                                    "opcode": "EventSemaphore",
                                    "sync_info": {"on_update": [], "on_wait": [w]},
                                }
                            )
                        si["on_wait"] = [waits[-1]]
                new_ins.append(ins)
            blk["instructions"] = new_ins
    return json.dumps(b).encode()


_PATCHED = False


def _install_bir_fixup():
    """Route the pjrt compile path's BIR bytes through _fixup_bir."""
    global _PATCHED
    if _PATCHED:
        return
    from concourse import bass2jax

    orig = bass2jax._decompress_ant_bir

    def patched(ant_bir_value):
        return _fixup_bir(orig(ant_bir_value))

    bass2jax._decompress_ant_bir = patched
    _PATCHED = True


def _rnorm(nc, pool, ss, tag):
    """ss [128, k] squared norms -> 1/max(sqrt(ss), eps) = exp(-0.5*ln(ss)).

    Ln+Exp live in the same activation table set as the main-loop Exp, so
    no ACT table reloads (Sqrt would force a set switch per group)."""
    ln = pool.tile(list(ss.shape), F32, tag=tag + "_ln")
    rn = pool.tile(list(ss.shape), F32, tag=tag + "_rn")
    nc.vector.tensor_scalar_max(out=ss, in0=ss, scalar1=EPS2)
    nc.scalar.activation(out=ln, in_=ss, func=mybir.ActivationFunctionType.Ln)
    nc.scalar.activation(
        out=rn, in_=ln, func=mybir.ActivationFunctionType.Exp, scale=-0.5
    )
    return rn


def _dot(nc, pool, a, b, accum_col):
    """accum_col [128,1] = sum over free dim of a*b (fp32), two DVE ops."""
    s = pool.tile([128, D], F32, tag="sq")
    nc.vector.tensor_mul(s, a, b)
    nc.vector.reduce_sum(out=accum_col, in_=s, axis=mybir.AxisListType.X)


def _emit(tc, nc, z_win, z_pos, ident_in, out, out_c):
    from contextlib import ExitStack

    Exp = mybir.ActivationFunctionType.Exp
    X = mybir.AxisListType.X

    with ExitStack() as ctx:
        singles = ctx.enter_context(tc.tile_pool(name="singles", bufs=1))
        zbig = ctx.enter_context(tc.tile_pool(name="zbig", bufs=2))
        znb = ctx.enter_context(tc.tile_pool(name="znb", bufs=40))
        sq = ctx.enter_context(tc.tile_pool(name="sq", bufs=3))
        st = ctx.enter_context(tc.tile_pool(name="st", bufs=6))
        esp = ctx.enter_context(tc.tile_pool(name="es", bufs=3))
        otp = ctx.enter_context(tc.tile_pool(name="ot", bufs=3))
        mmp = ctx.enter_context(tc.tile_pool(name="mmp", bufs=2, space="PSUM"))
        csp = ctx.enter_context(tc.tile_pool(name="csp", bufs=1, space="PSUM"))
        tpp = ctx.enter_context(tc.tile_pool(name="tpp", bufs=2, space="PSUM"))

        ident = singles.tile([128, 128], BF16)
        nc.sync.dma_start(out=ident, in_=ident_in[:, :])
        ones = singles.tile([128, 128], BF16)
        nc.vector.memset(ones, 1.0)

        znT0 = [
            singles.tile([128, GW], BF16, name=f"znT0_{i}", tag=f"znT0_{i}")
            for i in range(GB)
        ]
        znT1 = [
            singles.tile([128, GW], BF16, name=f"znT1_{i}", tag=f"znT1_{i}")
            for i in range(GB)
        ]
        diag = singles.tile([128, RT], F32)
        posd = singles.tile([128, RT], F32)
        rawp = singles.tile([128, RT], F32)
        rnl = singles.tile([128, RT], F32)
        racc = singles.tile([128, RT, CG], F32)
        csb = singles.tile([1, W], F32)

        def load_block(src):
            zb = zbig.tile([128, RT, D], F32, tag="zb")
            nc.sync.dma_start(out=zb, in_=src.rearrange("(k p) d -> p k d", p=128))
            return zb

        def transpose_group(zn_tiles, dst0, dst1):
            """16 [128, D] bf16 tiles -> dst0/dst1 [128, 2048] bf16 (d-chunk
            transposed layout) via PE transposes + DVE copies."""
            for half, dst in ((0, dst0), (1, dst1)):
                for q in range(2):
                    tp = tpp.tile([128, 1024], BF16, tag="tp")
                    for j in range(8):
                        t = q * 8 + j
                        nc.tensor.transpose(
                            tp[:, j * 128 : (j + 1) * 128],
                            zn_tiles[t][:, half * 128 : (half + 1) * 128],
                            ident,
                        )
                    nc.vector.tensor_copy(dst[:, q * 1024 : (q + 1) * 1024], tp)

        # ---- window: normalize + transpose, pipelined with the main loop
        zp = None
        for gb in range(GB):
            zf = load_block(z_win[gb * GW : (gb + 1) * GW, :])
            # rnorm in two 8-tile batches so the first transposes (and the
            # first matmuls) don't wait on the whole 16-tile group
            rnf_h = []
            for h in range(2):
                ssf = st.tile([128, RT // 2], F32, tag="ssf")
                for j in range(RT // 2):
                    t = h * (RT // 2) + j
                    _dot(nc, sq, zf[:, t, :], zf[:, t, :], ssf[:, j : j + 1])
                rnf_h.append(_rnorm(nc, st, ssf, "f"))
            znf = []
            for t in range(RT):
                rnf_col = rnf_h[t // (RT // 2)][:, t % (RT // 2) : t % (RT // 2) + 1]
                zb16 = znb.tile([128, D], BF16, tag="znf")
                nc.vector.tensor_scalar_mul(
                    out=zb16, in0=zf[:, t, :], scalar1=rnf_col
                )
                znf.append(zb16)
            transpose_group(znf, znT0[gb], znT1[gb])
            if gb == 0:
                # group 0 = the local rows: keep rnorm, diag dots (bf16,
                # PE-parity), raw pos dots (fp32). Emitted after the
                # transposes so the first matmuls start ASAP.
                nc.vector.tensor_copy(rnl[:, 0 : RT // 2], rnf_h[0])
                nc.vector.tensor_copy(rnl[:, RT // 2 : RT], rnf_h[1])
                zp = load_block(z_pos[:, :])
                ss_p = st.tile([128, RT], F32, tag="ss_p")
                for t in range(RT):
                    _dot(nc, sq, znf[t], znf[t], diag[:, t : t + 1])
                    _dot(nc, sq, zp[:, t, :], zp[:, t, :], ss_p[:, t : t + 1])
                    _dot(nc, sq, zf[:, t, :], zp[:, t, :], rawp[:, t : t + 1])
                rn_p = _rnorm(nc, st, ss_p, "p")

            # ---- main loop for the two 1024-col groups this gb provides
            for cg in (2 * gb, 2 * gb + 1):
                off = (cg * CW) % GW
                on_pe = True  # colsums on PE (GPSIMD can't write partition r)
                cs = csp.tile([128, CW], F32, tag="cs")
                for r in range(RT):
                    ps = mmp.tile([128, CW], F32, tag="ps")
                    for k in range(2):
                        lhsT = (znT0 if k == 0 else znT1)[0][
                            :, r * 128 : (r + 1) * 128
                        ]
                        rhsT = (znT0 if k == 0 else znT1)[gb]
                        for s in range(CW // SUB):
                            c0 = off + s * SUB
                            nc.tensor.matmul(
                                ps[:, s * SUB : (s + 1) * SUB],
                                lhsT=lhsT,
                                rhs=rhsT[:, c0 : c0 + SUB],
                                start=(k == 0),
                                stop=(k == 1),
                            )
                    es = esp.tile([128, CW], BF16, tag="es")
                    nc.scalar.activation(
                        out=es,
                        in_=ps,
                        func=Exp,
                        scale=TEMP_INV,
                        accum_out=racc[:, r, cg : cg + 1],
                    )
                    # column sums (the partner rows' exp sums, by symmetry)
                    # accumulate over the 16 row tiles in PSUM
                    for s in range(CW // SUB):
                        nc.tensor.matmul(
                            cs[:, s * SUB : (s + 1) * SUB],
                            lhsT=ones,
                            rhs=es[:, s * SUB : (s + 1) * SUB],
                            start=(r == 0),
                            stop=(r == RT - 1),
                        )
                nc.vector.tensor_copy(
                    csb[0:1, cg * CW : (cg + 1) * CW], cs[0:1, :]
                )

        # ---- finalize per-row outputs
        # posd = rawp * rnl * rn_p  (fp32 cosine of positive pairs)
        nc.vector.tensor_mul(posd, rawp, rnl)
        nc.vector.tensor_mul(posd, posd, rn_p)
        for r in range(RT):
            o = otp.tile([128, 4], F32)
            nc.vector.reduce_sum(out=o[:, 0:1], in_=racc[:, r, :], axis=X)
            nc.vector.tensor_copy(o[:, 1:2], diag[:, r : r + 1])
            nc.vector.tensor_copy(o[:, 2:3], posd[:, r : r + 1])
            nc.vector.memset(o[:, 3:4], 0.0)
            nc.sync.dma_start(out=out[r * 128 : (r + 1) * 128, :], in_=o)
        nc.sync.dma_start(out=out_c[:, :], in_=csb)


def build_program():
    if "nc" in _CACHE:
        return _CACHE["nc"]
    nc = bass.Bass()
    z_win = nc.declare_dram_parameter("z_win", [W, D], F32, isOutput=False)
    z_pos = nc.declare_dram_parameter("z_pos", [RPC, D], F32, isOutput=False)
    ident = nc.declare_dram_parameter("ident", [128, 128], BF16, isOutput=False)
    out = nc.declare_dram_parameter("out", [RPC, 4], F32, isOutput=True)
    out_c = nc.declare_dram_parameter("out_c", [1, W], F32, isOutput=True)
    with tile.TileContext(nc) as tc:
        _emit(tc, nc, z_win[:, :], z_pos[:, :], ident[:, :], out[:, :], out_c[:, :])
    _CACHE["nc"] = nc
    return nc


def make_in_maps(z):
    import ml_dtypes

    eye = np.eye(128, dtype=ml_dtypes.bfloat16)
    zz = np.concatenate([z, z], axis=0)  # easy wraparound slicing
    in_maps = []
    for c in range(NCORES):
        r0 = c * RPC
        p0 = (r0 + B) % N
        in_maps.append(
            {
                "z_win": np.ascontiguousarray(zz[r0 : r0 + W]),
                "z_pos": zz[p0 : p0 + RPC],
                "ident": eye,
            }
        )
    return in_maps


def finalize(row_outs, col_outs):
    """row_outs: per-core [RPC, 4]; col_outs: per-core [1, W] -> scalar loss."""
    import ml_dtypes

    o = np.concatenate(row_outs, axis=0).astype(np.float64)  # [N, 4]
    rowsum, diagd, posd = o[:, 0], o[:, 1], o[:, 2]
    expsum = rowsum.copy()
    for c in range(NCORES):
        idx = (c * RPC + np.arange(W)) % N
        np.add.at(expsum, idx, col_outs[c].reshape(-1).astype(np.float64))
    ediag = np.exp(TEMP_INV * diagd)
    ediag_bf16 = ediag.astype(np.float32).astype(ml_dtypes.bfloat16).astype(np.float64)
    expsum += np.exp(TEMP_INV * posd) - ediag - ediag_bf16
    lse = np.log(expsum)
    return np.float32(np.mean(lse - TEMP_INV * posd))


def _enable_axon_trace_hook():
    """Best-effort: register the NTFF profile hook that the image's antenv
    stub does not ship, and neuter the artifact upload (no bucket creds
    in this container). Only needed when profiling (BASS_TRACE=1)."""
    import sys
    import types

    try:
        from antenv import axon_hooks  # noqa: F401
    except ImportError:
        try:
            import antenv
            from trn_agent_boot.trn_boot import _ntff_profile_via_ctypes

            mod = types.ModuleType("antenv.axon_hooks")
            _hook = [None]
            mod.set_axon_ntff_profile_hook = lambda h: _hook.__setitem__(0, h)
            mod.get_axon_ntff_profile_hook = lambda: _hook[0]
            sys.modules["antenv.axon_hooks"] = mod
            antenv.axon_hooks = mod
            mod.set_axon_ntff_profile_hook(
                _ntff_profile_via_ctypes("/opt/axon/libaxon_pjrt.so")
            )
        except Exception as e:  # pragma: no cover
            print(f"trace hook setup failed: {e}")
    try:
        from concourse import bass_utils as _bu

        _bu.upload_artifacts = lambda tmpdir: f"local:{tmpdir}"
    except Exception:
        pass


def kernel(z_i, z_j, logit_scale_m=None, **_unused):
    global last_exec_time_ns, last_mean_exec_time_ns
    z_i = np.ascontiguousarray(np.asarray(z_i, dtype=np.float32))
    z_j = np.ascontiguousarray(np.asarray(z_j, dtype=np.float32))
    assert z_i.shape == (B, D) and z_j.shape == (B, D)
    z = np.concatenate([z_i, z_j], axis=0)

    nc = build_program()
    in_maps = make_in_maps(z)
    _install_bir_fixup()
    trace = bool(os.environ.get("BASS_TRACE"))
    if trace:
        _enable_axon_trace_hook()
    res = run_bass_kernel_spmd(nc, in_maps, list(range(NCORES)), trace=trace)
    last_exec_time_ns = res.exec_time_ns
    last_mean_exec_time_ns = res.mean_exec_time_ns
    row_outs = [res.results[c]["out"] for c in range(NCORES)]
    col_outs = [res.results[c]["out_c"] for c in range(NCORES)]
    return np.asarray(finalize(row_outs, col_outs), dtype=np.float32)


# revision 21
# speedup vs baseline: 2.2073x; 1.0014x over previous
"""NT-Xent loss kernel for 8 Trainium2 NeuronCores (Bass/Tile).

Symmetric data-parallel strategy (each unordered pair computed once):
  - host: z = concat(z_i, z_j) [16384, 256] f32. Core c receives z rotated by
    its row offset: rot_c[i] = z[(2048c + i) % 16384]. With that rotation the
    IR is identical across cores: local rows are rot rows [0, 2048) and the
    core's column window is rot rows [0, 8192) - each unordered pair {i, j}
    lands in exactly one core's (local rows x window) block (pairs at offset
    exactly 8192 - the positive pairs - are excluded and handled on host).
  - device (identical SPMD IR on all 8 cores):
      * normalize window rows in fp32 (sumsq on DVE, rnorm = exp(-ln/2) on
        ACT - same table set as the main Exp), cast bf16, transpose on the
        tensor engine into per-group zn^T tiles (group-pipelined with the
        main loop). zn^T group 0 doubles as the local lhsT.
      * main loop over 8 x 1024-col PSUM groups x 16 local row tiles:
        2x2 accumulated bf16 matmuls (K=256) -> one ACT Exp per tile with
        fused row-sum (accum_out) -> two ones-matmuls on the PE accumulate
        the block's column sums in PSUM across the 16 row tiles (these are
        the partner rows' sums, by symmetry).
      * per-row diagonal dot (bf16, matches the PE diagonal) and fp32
        positive-pair dot on DVE.
  - host (fp64): expsum[i] = own rowsum + the 4 covering cores' colsums
    + exp(10*pos_i) - exp(10*diag_i) - bf16(exp(10*diag_i));
    loss = mean(log(expsum) - 10*pos).
"""

import os
import numpy as np

try:
    import concourse.bass as bass
except ImportError:  # pragma: no cover
    import sys

    sys.path.insert(0, "/opt/trn_rl_repo")
    import concourse.bass as bass

import concourse.mybir as mybir
import concourse.tile as tile
from concourse.bass_utils import run_bass_kernel_spmd

F32 = mybir.dt.float32
BF16 = mybir.dt.bfloat16
FP8 = mybir.dt.float8e4

B = 8192
D = 256
N = 2 * B  # 16384
NCORES = 8
RPC = N // NCORES  # 2048 local rows per core
RT = RPC // 128  # 16 local row tiles
W = N // 2  # 8192-column window per core
GB = 4  # window load/transpose groups (16 tiles each)
GW = W // GB  # 2048 columns of znT per group tile
CG = 8  # main-loop column groups
CW = W // CG  # 1024 cols per PSUM group (2 banks)
SUB = 512  # matmul free dim (1 PSUM bank)
TEMP_INV = 10.0  # 1 / temperature
EPS2 = 1e-16  # cos eps^2 (clamp on squared norm)

# set by the last run when BASS_TRACE=1 (read by test.py)
last_exec_time_ns = None
last_mean_exec_time_ns = None

_CACHE = {}


def _fixup_bir(bir_bytes):
    """Adapt Tile-emitted BIR to this container's walrus build:
    - split instructions carrying >1 sync wait (walrus allows one per inst)
    - replace the raw-ISA EVENT_SEMAPHORE_RANGE_CLEAR (encoding mismatch)
      with per-semaphore sem-wr-imm zero writes."""
    import json

    b = json.loads(bir_bytes)
    for fn in b["functions"]:
        for blk in fn["blocks"]:
            new_ins = []
            for ins in blk["instructions"]:
                if (
                    ins.get("opcode") == "ISA"
                    and ins.get("op_name") == "EVENT_SEMAPHORE_RANGE_CLEAR"
                ):
                    d = ins["ant_dict"]
                    for s in range(d["range_first"], d["range_last"] + 1):
                        new_ins.append(
                            {
                                "debug": ins.get("debug", 0),
                                "engine": ins["engine"],
                                "ins": [],
                                "outs": [],
                                "name": f'{ins["name"]}_z{s}',
                                "opcode": "EventSemaphore",
                                "sync_info": {
                                    "on_update": [
                                        {
                                            "ant_name": f"zero_{s}",
                                            "id": s,
                                            "sync_type": "semaphore",
                                            "update_mode": "sem-wr-imm",
                                            "update_value": 0,
                                        }
                                    ],
                                    "on_wait": [],
                                },
                            }
                        )
                    continue
                si = ins.get("sync_info")
                if si:
                    waits = si.get("on_wait") or []
                    if len(waits) > 1:
                        for j, w in enumerate(waits[:-1]):
                            new_ins.append(
                                {
                                    "debug": ins.get("debug", 0),
                                    "engine": ins["engine"],
                                    "ins": [],
                                    "outs": [],
                                    "name": f'{ins["name"]}_w{j}',
                                    "opcode": "EventSemaphore",
                                    "sync_info": {"on_update": [], "on_wait": [w]},
                                }
                            )
                        si["on_wait"] = [waits[-1]]
                new_ins.append(ins)
            blk["instructions"] = new_ins
    return json.dumps(b).encode()


_PATCHED = False


def _install_bir_fixup():
    """Route the pjrt compile path's BIR bytes through _fixup_bir."""
    global _PATCHED
    if _PATCHED:
        return
    from concourse import bass2jax

    orig = bass2jax._decompress_ant_bir

    def patched(ant_bir_value):
        return _fixup_bir(orig(ant_bir_value))

    bass2jax._decompress_ant_bir = patched
    _PATCHED = True


def _rnorm(nc, pool, ss, tag):
    """ss [128, k] squared norms -> 1/max(sqrt(ss), eps) = exp(-0.5*ln(ss)).

    Ln+Exp live in the same activation table set as the main-loop Exp, so
    no ACT table reloads (Sqrt would force a set switch per group)."""
    ln = pool.tile(list(ss.shape), F32, tag=tag + "_ln")
    rn = pool.tile(list(ss.shape), F32, tag=tag + "_rn")
    nc.vector.tensor_scalar_max(out=ss, in0=ss, scalar1=EPS2)
    nc.scalar.activation(out=ln, in_=ss, func=mybir.ActivationFunctionType.Ln)
    nc.scalar.activation(
        out=rn, in_=ln, func=mybir.ActivationFunctionType.Exp, scale=-0.5
    )
    return rn


def _dot(nc, pool, a, b, accum_col):
    """accum_col [128,1] = sum over free dim of a*b (fp32), two DVE ops."""
    s = pool.tile([128, D], F32, tag="sq")
    nc.vector.tensor_mul(s, a, b)
    nc.vector.reduce_sum(out=accum_col, in_=s, axis=mybir.AxisListType.X)


def _emit(tc, nc, z_win, z_pos, ident_in, out, out_c):
    from contextlib import ExitStack

    Exp = mybir.ActivationFunctionType.Exp
    X = mybir.AxisListType.X

    with ExitStack() as ctx:
        singles = ctx.enter_context(tc.tile_pool(name="singles", bufs=1))
        zbig = ctx.enter_context(tc.tile_pool(name="zbig", bufs=2))
        znb = ctx.enter_context(tc.tile_pool(name="znb", bufs=40))
        sq = ctx.enter_context(tc.tile_pool(name="sq", bufs=3))
        st = ctx.enter_context(tc.tile_pool(name="st", bufs=6))
        esp = ctx.enter_context(tc.tile_pool(name="es", bufs=3))
        otp = ctx.enter_context(tc.tile_pool(name="ot", bufs=3))
        mmp = ctx.enter_context(tc.tile_pool(name="mmp", bufs=2, space="PSUM"))
        csp = ctx.enter_context(tc.tile_pool(name="csp", bufs=1, space="PSUM"))
        tpp = ctx.enter_context(tc.tile_pool(name="tpp", bufs=2, space="PSUM"))

        ident = singles.tile([128, 128], BF16)
        nc.sync.dma_start(out=ident, in_=ident_in[:, :])
        ones = singles.tile([128, 128], BF16)
        nc.vector.memset(ones, 1.0)

        znT = [
            singles.tile([128, 2, GW], FP8, name=f"znT_{i}", tag=f"znT_{i}")
            for i in range(GB)
        ]
        diag = singles.tile([128, RT], F32)
        posd = singles.tile([128, RT], F32)
        rawp = singles.tile([128, RT], F32)
        rnl = singles.tile([128, RT], F32)
        racc = singles.tile([128, RT, CG], F32)
        csb = singles.tile([1, W], F32)

        def load_block(src):
            zb = zbig.tile([128, RT, D], F32, tag="zb")
            nc.sync.dma_start(out=zb, in_=src.rearrange("(k p) d -> p k d", p=128))
            return zb

        def transpose_group(zn_tiles, dst):
            """16 [128, D] bf16 tiles -> dst [128, 2, 2048] fp8 (d-half on the
            middle dim - the DoubleRow K layout). PE transposes run in bf16
            (fp8 transpose needs stride-2 PSUM writes); the DVE copy casts
            bf16 -> fp8e4."""
            for half in (0, 1):
                for q in range(2):
                    tp = tpp.tile([128, 1024], BF16, tag="tp")
                    for j in range(8):
                        t = q * 8 + j
                        nc.tensor.transpose(
                            tp[:, j * 128 : (j + 1) * 128],
                            zn_tiles[t][:, half * 128 : (half + 1) * 128],
                            ident,
                        )
                    nc.vector.tensor_copy(
                        dst[:, half, q * 1024 : (q + 1) * 1024], tp
                    )

        # ---- window: normalize + transpose, pipelined with the main loop
        zp = None
        for gb in range(GB):
            zf = load_block(z_win[gb * GW : (gb + 1) * GW, :])
            # rnorm in two 8-tile batches so the first transposes (and the
            # first matmuls) don't wait on the whole 16-tile group
            rnf_h = []
            for h in range(2):
                ssf = st.tile([128, RT // 2], F32, tag="ssf")
                for j in range(RT // 2):
                    t = h * (RT // 2) + j
                    _dot(nc, sq, zf[:, t, :], zf[:, t, :], ssf[:, j : j + 1])
                rnf_h.append(_rnorm(nc, st, ssf, "f"))
            znf = []
            for t in range(RT):
                rnf_col = rnf_h[t // (RT // 2)][:, t % (RT // 2) : t % (RT // 2) + 1]
                zb16 = znb.tile([128, D], BF16, tag="znf")
                nc.vector.tensor_scalar_mul(
                    out=zb16, in0=zf[:, t, :], scalar1=rnf_col
                )
                znf.append(zb16)
            transpose_group(znf, znT[gb])
            if gb == 0:
                # group 0 = the local rows: keep rnorm, diag dots (bf16,
                # PE-parity), raw pos dots (fp32). Emitted after the
                # transposes so the first matmuls start ASAP.
                nc.vector.tensor_copy(rnl[:, 0 : RT // 2], rnf_h[0])
                nc.vector.tensor_copy(rnl[:, RT // 2 : RT], rnf_h[1])
                zp = load_block(z_pos[:, :])
                ss_p = st.tile([128, RT], F32, tag="ss_p")
                for t in range(RT):
                    zf8 = znb.tile([128, D], FP8, tag="zf8")
                    nc.vector.tensor_copy(zf8, znf[t])
                    _dot(nc, sq, zf8, zf8, diag[:, t : t + 1])
                    _dot(nc, sq, zp[:, t, :], zp[:, t, :], ss_p[:, t : t + 1])
                    _dot(nc, sq, zf[:, t, :], zp[:, t, :], rawp[:, t : t + 1])
                rn_p = _rnorm(nc, st, ss_p, "p")

            # ---- main loop for the two 1024-col groups this gb provides
            for cg in (2 * gb, 2 * gb + 1):
                off = (cg * CW) % GW
                on_pe = True  # colsums on PE (GPSIMD can't write partition r)
                cs = csp.tile([128, CW], F32, tag="cs")
                for r in range(RT):
                    ps = mmp.tile([128, CW], F32, tag="ps")
                    lhsT = znT[0][:, :, r * 128 : (r + 1) * 128]
                    for s in range(CW // SUB):
                        c0 = off + s * SUB
                        nc.tensor.matmul(
                            ps[:, s * SUB : (s + 1) * SUB],
                            lhsT=lhsT,
                            rhs=znT[gb][:, :, c0 : c0 + SUB],
                            start=True,
                            stop=True,
                            perf_mode=mybir.MatmulPerfMode.DoubleRow,
                        )
                    es = esp.tile([128, CW], BF16, tag="es")
                    nc.scalar.activation(
                        out=es,
                        in_=ps,
                        func=Exp,
                        scale=TEMP_INV,
                        accum_out=racc[:, r, cg : cg + 1],
                    )
                    # column sums (the partner rows' exp sums, by symmetry)
                    # accumulate over the 16 row tiles in PSUM
                    for s in range(CW // SUB):
                        nc.tensor.matmul(
                            cs[:, s * SUB : (s + 1) * SUB],
                            lhsT=ones,
                            rhs=es[:, s * SUB : (s + 1) * SUB],
                            start=(r == 0),
                            stop=(r == RT - 1),
                        )
                nc.vector.tensor_copy(
                    csb[0:1, cg * CW : (cg + 1) * CW], cs[0:1, :]
                )

        # ---- finalize per-row outputs
        # posd = rawp * rnl * rn_p  (fp32 cosine of positive pairs)
        nc.vector.tensor_mul(posd, rawp, rnl)
        nc.vector.tensor_mul(posd, posd, rn_p)
        for r in range(RT):
            o = otp.tile([128, 4], F32)
            nc.vector.reduce_sum(out=o[:, 0:1], in_=racc[:, r, :], axis=X)
            nc.vector.tensor_copy(o[:, 1:2], diag[:, r : r + 1])
            nc.vector.tensor_copy(o[:, 2:3], posd[:, r : r + 1])
            nc.vector.memset(o[:, 3:4], 0.0)
            nc.sync.dma_start(out=out[r * 128 : (r + 1) * 128, :], in_=o)
        nc.sync.dma_start(out=out_c[:, :], in_=csb)


def build_program():
    if "nc" in _CACHE:
        return _CACHE["nc"]
    nc = bass.Bass()
    z_win = nc.declare_dram_parameter("z_win", [W, D], F32, isOutput=False)
    z_pos = nc.declare_dram_parameter("z_pos", [RPC, D], F32, isOutput=False)
    ident = nc.declare_dram_parameter("ident", [128, 128], BF16, isOutput=False)
    out = nc.declare_dram_parameter("out", [RPC, 4], F32, isOutput=True)
    out_c = nc.declare_dram_parameter("out_c", [1, W], F32, isOutput=True)
    with tile.TileContext(nc) as tc:
        _emit(tc, nc, z_win[:, :], z_pos[:, :], ident[:, :], out[:, :], out_c[:, :])
    _CACHE["nc"] = nc
    return nc


def make_in_maps(z):
    import ml_dtypes

    eye = np.eye(128, dtype=ml_dtypes.bfloat16)
    zz = np.concatenate([z, z], axis=0)  # easy wraparound slicing
    in_maps = []
    for c in range(NCORES):
        r0 = c * RPC
        p0 = (r0 + B) % N
        in_maps.append(
            {
                "z_win": np.ascontiguousarray(zz[r0 : r0 + W]),
                "z_pos": zz[p0 : p0 + RPC],
                "ident": eye,
            }
        )
    return in_maps


def finalize(row_outs, col_outs):
    """row_outs: per-core [RPC, 4]; col_outs: per-core [1, W] -> scalar loss."""
    import ml_dtypes

    o = np.concatenate(row_outs, axis=0).astype(np.float64)  # [N, 4]
    rowsum, diagd, posd = o[:, 0], o[:, 1], o[:, 2]
    expsum = rowsum.copy()
    for c in range(NCORES):
        idx = (c * RPC + np.arange(W)) % N
        np.add.at(expsum, idx, col_outs[c].reshape(-1).astype(np.float64))
    ediag = np.exp(TEMP_INV * diagd)
    ediag_bf16 = ediag.astype(np.float32).astype(ml_dtypes.bfloat16).astype(np.float64)
    expsum += np.exp(TEMP_INV * posd) - ediag - ediag_bf16
    lse = np.log(expsum)
    return np.float32(np.mean(lse - TEMP_INV * posd))


def _enable_axon_trace_hook():
    """Best-effort: register the NTFF profile hook that the image's antenv
    stub does not ship, and neuter the artifact upload (no bucket creds
    in this container). Only needed when profiling (BASS_TRACE=1)."""
    import sys
    import types

    try:
        from antenv import axon_hooks  # noqa: F401
    except ImportError:
        try:
            import antenv
            from trn_agent_boot.trn_boot import _ntff_profile_via_ctypes

            mod = types.ModuleType("antenv.axon_hooks")
            _hook = [None]
            mod.set_axon_ntff_profile_hook = lambda h: _hook.__setitem__(0, h)
            mod.get_axon_ntff_profile_hook = lambda: _hook[0]
            sys.modules["antenv.axon_hooks"] = mod
            antenv.axon_hooks = mod
            mod.set_axon_ntff_profile_hook(
                _ntff_profile_via_ctypes("/opt/axon/libaxon_pjrt.so")
            )
        except Exception as e:  # pragma: no cover
            print(f"trace hook setup failed: {e}")
    try:
        from concourse import bass_utils as _bu

        _bu.upload_artifacts = lambda tmpdir: f"local:{tmpdir}"
    except Exception:
        pass


def kernel(z_i, z_j, logit_scale_m=None, **_unused):
    global last_exec_time_ns, last_mean_exec_time_ns
    z_i = np.ascontiguousarray(np.asarray(z_i, dtype=np.float32))
    z_j = np.ascontiguousarray(np.asarray(z_j, dtype=np.float32))
    assert z_i.shape == (B, D) and z_j.shape == (B, D)
    z = np.concatenate([z_i, z_j], axis=0)

    nc = build_program()
    in_maps = make_in_maps(z)
    _install_bir_fixup()
    trace = bool(os.environ.get("BASS_TRACE"))
    if trace:
        _enable_axon_trace_hook()
    res = run_bass_kernel_spmd(nc, in_maps, list(range(NCORES)), trace=trace)
    last_exec_time_ns = res.exec_time_ns
    last_mean_exec_time_ns = res.mean_exec_time_ns
    row_outs = [res.results[c]["out"] for c in range(NCORES)]
    col_outs = [res.results[c]["out_c"] for c in range(NCORES)]
    return np.asarray(finalize(row_outs, col_outs), dtype=np.float32)
